# revision 34
# baseline (speedup 1.0000x reference)
"""DREAMReconstructor Trainium2 kernel.

Strategy: data-parallel over batch across 8 NeuronCores (8 rows/core).
Per core, a fully-unrolled 256-step recurrence where every matmul keeps
batch (M=8) as the PE stationary-column dim.

The recurrence is mildly chaotic (errors double every ~16-20 steps), so
every matmul on the h/err/A feedback path must stay fp32 (LOW_HIGH);
fp32r (~1.6e-4) and bf16 (~2.8e-3) injections blow past the 2e-2 budget
after ~5000x amplification. Only the decoder/recon quadrant (pure
output, no feedback) runs in bf16.

Structure (software-pipelined emission; loop body t emits the tail of
step t interleaved with the head of step t+1 so the PE FIFO matches the
pipeline):
  - wave: per K-chunk "trios" (xhat q0 / rec-lo q32 / rec-hi q64) run
    concurrently in separate PE col groups, gated by the h'T ladder;
    dec (q96, bf16) and the fast-weight readout gmm overlap the trios
    in col group 3,
  - fast-weight readout gmm(t) = kappa*A(t-2)@k(t) is UNSCALED and goes
    to its own psum bank (PG); the decay scale sa(t-1) and the one-step
    staleness correction c8*h (kdot = 0.25*<k_t,k_{t-1}> precomputed on
    device) are folded in off-critical via u/v; tanh input is then one
    TT add: tm = v + pre_psum. This keeps the sigmoid->sa chain off the
    wave critical path,
  - rec psum regions are opened by dependency-free zero bf16 matmuls
    (transpose-mode start=True at nonzero col positions is broken), so
    the xin transpose-injections run early in the scalar window,
  - h'T ladder: per-chunk [40,128]@D40 matmuls + psum->SBUF copies
    interleaved with the next wave's trios; batch-major h' via DVE
    (om*h + iv*tanh),
  - A outer product (pa) + A_sb decay-update are emitted after the next
    wave so they execute in the PE-idle scalar window; per-step small
    scalars live in double-buffered per-step tiles to avoid cross-step
    WAR stalls; GpSimd (Pool) only runs tiny/early ops (it is ~1 DSP
    per 16 partitions -- never give it few-partition wide ops),
  - HAM keep-warm: two tiny dependency-free bf16 matmuls fill the PE
    idle window so the clock gate stays at 8/8.

History: baseline 4.35ms -> this version ~3.38ms on hardware.
"""
import sys
import numpy as np

sys.path.insert(0, "/opt/trn_rl_repo")

import concourse.bass as bass
import concourse.tile as tile
from concourse import bacc, mybir
from concourse.bass_utils import run_bass_kernel_spmd

F32 = mybir.dt.float32
BF16 = mybir.dt.bfloat16

B, T, D, H, R = 64, 256, 256, 512, 16
NCORES = 8
BL = B // NCORES  # 8 rows per core

FORGET = 0.005
BASE_PLAST = 0.5
BASE_THRESH = 0.3
SURP_TEMP = 0.05
ERR_SMOOTH = 0.05
LTC_TAU = 5.0
LTC_SCALE = 5.0
KAPPA = 0.5
SLEEP_RATE = 0.01
MIN_SURP = 0.15

_CACHE = {}


def _host_constants(W_in, W_rec, W_pred, B_proj, W_dec, b_dec):
    bf = mybir.dt.np(BF16)
    c = {}
    W_rp = np.concatenate([W_rec, W_pred], axis=0)                   # [768, 512]
    c["W_allT"] = np.ascontiguousarray(
        W_rp.T.reshape(512, 768).reshape(4, 128, 768))               # fp32
    c["WdecB"] = np.ascontiguousarray(
        W_dec.T.reshape(512, 256).reshape(4, 128, 256)).astype(bf)   # bf16
    c["W_inT"] = np.ascontiguousarray(W_in.T.reshape(2, 128, H))     # [256,512]->[2,128,512]
    c["BprojT"] = np.ascontiguousarray(B_proj.T.reshape(2, 128, R))  # [256,16]->[2,128,16]
    c["I128"] = np.eye(128, dtype=np.float32)
    e = np.zeros((128, 8), np.float32)
    for p in range(128):
        e[p, p // 16] = KAPPA                                        # kappa folded into readout
    c["E_half"] = e
    c["E_q"] = np.ascontiguousarray(0.5 * e)                         # kappa*plast halves
    c["E_bT"] = np.ascontiguousarray((e.T != 0).astype(np.float32))  # [8,128] 0/1
    es = np.zeros((40, 8), np.float32)
    eo = np.zeros((40, 8), np.float32)
    for b in range(8):
        es[b, b] = -1.0          # h_b side: om = 0.8 - surp
        eo[b, b] = 0.8
        es[32 + b, b] = 1.0      # th side: iv = 0.2 + surp
        eo[32 + b, b] = 0.2
    c["Esign40"] = es
    c["Eoff40"] = eo
    return c


def _build(nsteps):
    nc = bacc.Bacc("TRN2", target_bir_lowering=False, debug=False, num_devices=NCORES)

    d_xT = nc.dram_tensor("xT", [128, 2, T, BL], F32, kind="ExternalInput")
    d_WallT = nc.dram_tensor("W_allT", [4, 128, 768], F32, kind="ExternalInput")
    d_WdB = nc.dram_tensor("WdecB", [4, 128, 256], BF16, kind="ExternalInput")
    d_WinT = nc.dram_tensor("W_inT", [2, 128, H], F32, kind="ExternalInput")
    d_BpT = nc.dram_tensor("BprojT", [2, 128, R], F32, kind="ExternalInput")
    d_I = nc.dram_tensor("I128", [128, 128], F32, kind="ExternalInput")
    d_nxT = nc.dram_tensor("nxT", [128, 2, T, BL], F32, kind="ExternalInput")
    d_Eh = nc.dram_tensor("E_half", [128, 8], F32, kind="ExternalInput")
    d_Eq = nc.dram_tensor("E_q", [128, 8], F32, kind="ExternalInput")
    d_EbT = nc.dram_tensor("E_bT", [8, 128], F32, kind="ExternalInput")
    d_Es = nc.dram_tensor("Esign40", [40, 8], F32, kind="ExternalInput")
    d_Eo = nc.dram_tensor("Eoff40", [40, 8], F32, kind="ExternalInput")
    d_y = nc.dram_tensor("y", [BL, T, D], F32, kind="ExternalOutput")
    import os
    DBG = os.environ.get("KDBG", "0") == "1"
    if DBG:
        d_dbg = nc.dram_tensor("dbg", [8, 6 * 256], F32, kind="ExternalOutput")

    AL = mybir.AluOpType
    AF = mybir.ActivationFunctionType

    from concourse.tile_rust import add_dep_helper

    _pe_prev = [None]

    with tile.TileContext(nc) as tc:
        def MM(*args, **kw):
            return nc.tensor.matmul(*args, **kw)

        with tc.tile_pool(name="persist", bufs=1) as P:
            # persistent tiles
            WallT = [P.tile([128, 768], F32, tag=f"WallT{i}", name=f"WallT{i}") for i in range(4)]
            for kc in range(4):
                nc.sync.dma_start(WallT[kc][:], d_WallT[kc])
            WdB = [P.tile([128, 256], BF16, tag=f"WdB{i}", name=f"WdB{i}") for i in range(4)]
            for kc in range(4):
                nc.sync.dma_start(WdB[kc][:], d_WdB[kc])
            I128 = P.tile([128, 128], F32); nc.sync.dma_start(I128[:], d_I[:])
            E_half = P.tile([128, 8], F32); nc.sync.dma_start(E_half[:], d_Eh[:])
            E_q = P.tile([128, 8], F32); nc.sync.dma_start(E_q[:], d_Eq[:])
            E_bT = P.tile([8, 128], F32); nc.sync.dma_start(E_bT[:], d_EbT[:])
            Es40 = P.tile([40, 8], F32); nc.sync.dma_start(Es40[:], d_Es[:])
            Eo40 = P.tile([40, 8], F32); nc.sync.dma_start(Eo40[:], d_Eo[:])

            nxT = P.tile([128, 2 * T * BL], F32)
            nxT_v = nxT[:].rearrange("p (dc t b) -> p dc t b", dc=2, t=T, b=BL)
            nc.sync.dma_start(nxT_v[:, :, :, :], d_nxT[:, :, :, :])

            xinT = P.tile([128, 4 * T * BL], F32)
            xinT_v = xinT[:].rearrange("p (hc t b) -> p hc t b", hc=4, t=T, b=BL)
            K_all = P.tile([128, T], F32)
            A_sb = P.tile([128, H], F32)
            hT = P.tile([128, 32], F32)
            hT_b = P.tile([128, 32], BF16)
            hh = P.tile([40, H], F32)
            nE20 = P.tile([8, 1], F32)
            sa8_init = P.tile([8, 1], F32)
            sp2_init = P.tile([8, 1], F32)
            kdot = P.tile([8, T], F32)
            c_nthr = P.tile([8, 1], F32)

            nc.vector.memset(A_sb[:], 0.0)
            nc.vector.memset(hT[:], 0.0)
            nc.vector.memset(hT_b[:], 0.0)
            nc.vector.memset(hh[:], 0.0)
            nc.vector.memset(nE20[:], 0.0)
            nc.vector.memset(c_nthr[:], float(-BASE_THRESH))
            c_one = P.tile([8, 1], F32)
            nc.vector.memset(c_one[:], 1.0)
            c_995 = P.tile([8, 1], F32)
            nc.vector.memset(c_995[:], float(1.0 - FORGET))
            nc.vector.memset(sa8_init[:], 1.0)
            nc.vector.memset(sp2_init[:], 0.0)
            nc.vector.memset(kdot[:], 0.0)
            z8b = P.tile([128, 8], BF16)
            nc.vector.memset(z8b[:], 0.0)

            # ---------------- precompute: xin = x @ W_in.T, k = x @ B_proj.T
            with tc.tile_pool(name="pre_sb", bufs=3) as PS, \
                 tc.tile_pool(name="pre_ps", bufs=2, space="PSUM") as PP:
                WinT = [PS.tile([128, H], F32, tag=f"WinT{i}", name=f"WinT{i}") for i in range(2)]
                BpT = [PS.tile([128, R], F32, tag=f"BpT{i}", name=f"BpT{i}") for i in range(2)]
                for dc in range(2):
                    nc.sync.dma_start(WinT[dc][:], d_WinT[dc])
                    nc.sync.dma_start(BpT[dc][:], d_BpT[dc])
                TB = 64  # t-block
                for tb in range(T // TB):
                    rhs = [PS.tile([128, TB * BL], F32, tag=f"xrhs{i}", name=f"xrhs{i}") for i in range(2)]
                    for dc in range(2):
                        nc.sync.dma_start(
                            rhs[dc][:],
                            d_xT[:, dc, tb * TB:(tb + 1) * TB, :])
                    for hc in range(4):
                        ps = PP.tile([128, TB * BL], F32, tag="xps")
                        MM(ps[:], WinT[0][:, 128 * hc:128 * (hc + 1)],
                                         rhs[0][:], start=True, stop=False)
                        MM(ps[:], WinT[1][:, 128 * hc:128 * (hc + 1)],
                                         rhs[1][:], start=False, stop=True)
                        if hc % 2 == 0:
                            nc.vector.tensor_copy(
                                xinT_v[:, hc, tb * TB:(tb + 1) * TB, :], ps[:])
                        else:
                            nc.scalar.copy(
                                xinT_v[:, hc, tb * TB:(tb + 1) * TB, :], ps[:])
                    psk = PP.tile([128, TB * BL], F32, tag="xps")
                    MM(psk[0:16, :], BpT[0][:], rhs[0][:],
                                     start=True, stop=False)
                    MM(psk[0:16, :], BpT[1][:], rhs[1][:],
                                     start=False, stop=True)
                    kb = PS.tile([16, TB * BL], F32, tag="kb")
                    nc.vector.tensor_copy(kb[:], psk[0:16, :])
                    kb_v = kb[:].rearrange("r (t b) -> r t b", t=TB, b=BL)
                    for b in range(8):
                        nc.sync.dma_start(
                            K_all[16 * b:16 * (b + 1), tb * TB:(tb + 1) * TB],
                            kb_v[:, :, b])

                # kdot[b, t] = 0.5 * <k_t, k_{t-1}> (E_half carries the 0.5)
                prodk = PS.tile([128, T - 1], F32, tag="prodk")
                nc.vector.tensor_tensor(prodk[:], K_all[:, 1:T], K_all[:, 0:T - 1],
                                        AL.mult)
                psd = PP.tile([128, T - 1], F32, tag="xps")
                MM(psd[0:8, :], E_q[:], prodk[:],
                                 start=True, stop=True)
                nc.vector.tensor_copy(kdot[:, 1:T], psd[0:8, :])

            # ---------------- recurrent loop
            with tc.tile_pool(name="pm", bufs=2, space="PSUM") as PM, \
                 tc.tile_pool(name="pa", bufs=2, space="PSUM") as PA, \
                 tc.tile_pool(name="ph", bufs=2, space="PSUM") as PH, \
                 tc.tile_pool(name="pg", bufs=2, space="PSUM") as PGP, \
                 tc.tile_pool(name="step", bufs=2) as SP:

                def emit_ksel(tt):
                    """K_sel(tt) = E_half * k(tt), shared by the step-(tt)
                    A-outer-product and the step-(tt) fast readout."""
                    ks = SP.tile([128, 8], F32, tag="ksel", name=f"ksel{tt}")
                    nc.vector.tensor_scalar(ks[:], E_half[:],
                                            K_all[:, tt:tt + 1], None, AL.mult)
                    return ks

                def emit_gmm(tt, pg_t, ks):
                    """Unscaled old-A readout kappa*A_sb@k(tt) into its own
                    psum bank at col-group 96 (decay scaling applied at the
                    pre-tanh merge). Reads A_sb BEFORE the step-(tt-1) update."""
                    MM(pg_t[96:104, 0:256], ks[:], A_sb[:, 0:256],
                                     start=True, stop=True, tile_position=(0, 96))
                    MM(pg_t[96:104, 256:512], ks[:], A_sb[:, 256:512],
                                     start=True, stop=True, tile_position=(0, 96))

                def emit_head(tt):
                    """pm(tt) + ready-early injectors: -x transposes (open
                    xhat), zero-openers (open rec regions), xin adds."""
                    pm_t = PM.tile([128, 512], F32, tag="pm", name=f"pm{tt}")
                    for dc in range(2):
                        MM(
                            pm_t[0:8, 128 * dc:128 * (dc + 1)],
                            nxT_v[:, dc, tt, :], I128[:], is_transpose=True,
                            start=(dc == 0), stop=False, tile_position=(0, 0))
                    MM(pm_t[32:40, 0:256], z8b[:], WdB[0][:],
                                     start=True, stop=False,
                                     tile_position=(0, 32))
                    MM(pm_t[64:72, 0:256], z8b[:], WdB[0][:],
                                     start=True, stop=False,
                                     tile_position=(0, 64))
                    for cx in range(4):
                        j = 1 + cx // 2
                        MM(
                            pm_t[32 * j:32 * j + 8,
                                 128 * (cx % 2):128 * (cx % 2 + 1)],
                            xinT_v[:, cx, tt, :], I128[:],
                            start=False, stop=False, tile_position=(0, 32 * j))
                    return pm_t

                def emit_trio(pm_t, kc):
                    """One K-chunk of the recurrent wave: xhat/rec-lo/rec-hi
                    fp32 (kc==3 closes the regions)."""
                    MM(pm_t[0:8, 0:256],
                                     hT[:, 8 * kc:8 * (kc + 1)],
                                     WallT[kc][:, 512:768],
                                     start=False, stop=(kc == 3),
                                     tile_position=(0, 0))
                    MM(pm_t[32:40, 0:256],
                                     hT[:, 8 * kc:8 * (kc + 1)],
                                     WallT[kc][:, 0:256],
                                     start=False, stop=(kc == 3),
                                     tile_position=(0, 32))
                    MM(pm_t[64:72, 0:256],
                                     hT[:, 8 * kc:8 * (kc + 1)],
                                     WallT[kc][:, 256:512],
                                     start=False, stop=(kc == 3),
                                     tile_position=(0, 64))

                def emit_dec(pm_t, kc):
                    MM(pm_t[96:104, 0:256],
                                     hT_b[:, 8 * kc:8 * (kc + 1)],
                                     WdB[kc][:],
                                     start=(kc == 0), stop=(kc == 3),
                                     tile_position=(0, 96))

                def emit_uv(tt, pg_t, sa8, sp2):
                    """c-term + scaled stale readout for step tt (all early,
                    off the rec-close critical path)."""
                    c8 = SP.tile([8, 1], F32, tag="c8", name=f"c8_{tt}")
                    nc.gpsimd.tensor_tensor(c8[:], kdot[:, tt:tt + 1], sp2[:],
                                            AL.mult)
                    u1 = SP.tile([8, 256], F32, tag="u1")
                    u2 = SP.tile([8, 256], F32, tag="u2")
                    nc.vector.tensor_scalar(u1[:], pg_t[96:104, 0:256], sa8[:],
                                            None, AL.mult)
                    nc.vector.tensor_scalar(u2[:], pg_t[96:104, 256:512],
                                            sa8[:], None, AL.mult)
                    v1 = SP.tile([8, 256], F32, tag="v1", name=f"v1_{tt}")
                    v2 = SP.tile([8, 256], F32, tag="v2", name=f"v2_{tt}")
                    nc.vector.scalar_tensor_tensor(v1[:], hh[0:8, 0:256], c8[:],
                                                   u1[:], AL.mult, AL.add)
                    nc.vector.scalar_tensor_tensor(v2[:], hh[0:8, 256:512],
                                                   c8[:], u2[:],
                                                   AL.mult, AL.add)
                    return v1, v2

                # -------- prologue: step 0 head + wave on h=0
                ksel_cur = emit_ksel(0)
                pg_cur = PGP.tile([128, 512], F32, tag="pg", name="pg_pro")
                emit_gmm(0, pg_cur, ksel_cur)
                pm_cur = emit_head(0)
                for kc in range(4):
                    emit_trio(pm_cur, kc)
                for kc in range(4):
                    emit_dec(pm_cur, kc)
                v1_cur, v2_cur = emit_uv(0, pg_cur, sa8_init, sp2_init)

                for t in range(nsteps):
                    pm = pm_cur
                    pg = pg_cur
                    ksel = ksel_cur
                    v1, v2 = v1_cur, v2_cur
                    last = t == nsteps - 1

                    pa = PA.tile([128, 512], F32, tag="pa")
                    ph = PH.tile([128, 64], F32, tag="ph")

                    # HAM keep-warm at body start: fills the PE-idle window
                    # between wave-close and the h'T ladder
                    for dw in range(2):
                        MM(ph[0:8, 48 + 8 * dw:56 + 8 * dw], z8b[:],
                           WdB[1][:, 0:8], start=True, stop=True,
                           tile_position=(0, 0))

                    # ---- error chain (after xhat q0 closes); all smalls
                    # are per-step tiles so step t+1 never WAR-stalls on them
                    sqd = SP.tile([8, 256], F32, tag="sqd")
                    errsum = SP.tile([8, 1], F32, tag="errsum")
                    surp40 = SP.tile([40, 1], F32, tag="surp40")
                    om8 = SP.tile([8, 1], F32, tag="om8")
                    iv40 = SP.tile([40, 1], F32, tag="iv40")
                    tmp8 = SP.tile([8, 1], F32, tag="tmp8")
                    r8 = SP.tile([8, 1], F32, tag="r8")
                    lt8 = SP.tile([8, 1], F32, tag="lt8")
                    sleep8 = SP.tile([8, 1], F32, tag="sleep8")
                    sa8 = SP.tile([8, 1], F32, tag="sa8", name=f"sa8_{t}")
                    sp2 = SP.tile([8, 1], F32, tag="sp2", name=f"sp2_{t}")
                    nc.scalar.activation(sqd[:], pm[0:8, 0:256], AF.Square,
                                         accum_out=errsum[:])
                    nc.scalar.activation(surp40[0:8, :], errsum[:],
                                         AF.Sigmoid, bias=nE20[:],
                                         scale=float(1.0 / (256 * SURP_TEMP)))
                    nc.scalar.activation(surp40[32:40, :], errsum[:],
                                         AF.Sigmoid, bias=nE20[:],
                                         scale=float(1.0 / (256 * SURP_TEMP)))
                    D40 = SP.tile([40, 8], F32, tag="d40")
                    nc.vector.scalar_tensor_tensor(D40[:], Es40[:], surp40[:],
                                                   Eo40[:], AL.mult, AL.add)
                    nc.gpsimd.tensor_scalar(om8[:], surp40[0:8, :], -1.0, 0.8,
                                            AL.mult, AL.add)
                    nc.gpsimd.tensor_scalar(iv40[32:40, :], surp40[32:40, :],
                                            0.2, None, AL.add)
                    nc.gpsimd.tensor_scalar(tmp8[:], errsum[:],
                                            float(-1.0 / 256.0), None, AL.mult)
                    nc.vector.scalar_tensor_tensor(nE20[:], nE20[:],
                                                   float(1.0 - ERR_SMOOTH),
                                                   tmp8[:], AL.mult, AL.add)
                    nc.scalar.activation(r8[:], surp40[0:8, :], AF.Relu,
                                         bias=c_nthr[:])
                    nc.vector.tensor_scalar(lt8[:], surp40[0:8, :],
                                            float(MIN_SURP), None, AL.is_lt)
                    nc.scalar.activation(sleep8[:], lt8[:], AF.Relu,
                                         bias=c_one[:], scale=float(-SLEEP_RATE))
                    nc.scalar.activation(sa8[:], lt8[:], AF.Relu,
                                         bias=c_995[:],
                                         scale=float(-SLEEP_RATE * (1.0 - FORGET)))
                    nc.gpsimd.tensor_tensor(sp2[:], r8[:], sleep8[:], AL.mult)
                    # HAM keep-warm: dependency-free bf16 matmuls that fill
                    # the PE-idle scalar window so the clock stays at 8/8
                    for dw in range(2):
                        MM(ph[0:8, 40 + 8 * dw:48 + 8 * dw], z8b[:],
                           WdB[0][:, 0:8], start=True, stop=True,
                           tile_position=(0, 0))

                    # ---- recon_{t-1} out
                    if t > 0:
                        rec_sb = SP.tile([8, 256], F32, tag="rec", bufs=3)
                        if t % 2 == 0:
                            nc.scalar.copy(rec_sb[:], pm[96:104, 0:256])
                        else:
                            nc.vector.tensor_copy(rec_sb[:], pm[96:104, 0:256])
                        nc.sync.dma_start(d_y[:, t - 1, :], rec_sb[:])

                    # ---- tanh inputs + tanh
                    tm1 = SP.tile([8, 256], F32, tag="tm1")
                    tm2 = SP.tile([8, 256], F32, tag="tm2")
                    nc.vector.tensor_tensor(tm1[:], v1[:], pm[32:40, 0:256],
                                            AL.add)
                    nc.scalar.activation(hh[32:40, 0:256], tm1[:], AF.Tanh)
                    nc.vector.tensor_tensor(tm2[:], v2[:], pm[64:72, 0:256],
                                            AL.add)
                    nc.scalar.activation(hh[32:40, 256:512], tm2[:], AF.Tanh)

                    # ---- h'T ladder interleaved with next step's wave
                    def emit_ph(kc):
                        MM(ph[:, 8 * kc:8 * (kc + 1)],
                                         hh[:, 128 * kc:128 * (kc + 1)], D40[:],
                                         start=True, stop=True)
                        if kc % 2 == 0:
                            nc.scalar.copy(hT[:, 8 * kc:8 * (kc + 1)],
                                           ph[:, 8 * kc:8 * (kc + 1)])
                        else:
                            nc.vector.tensor_copy(hT[:, 8 * kc:8 * (kc + 1)],
                                                  ph[:, 8 * kc:8 * (kc + 1)])

                    emit_ph(0)
                    emit_ph(1)
                    if not last:
                        ksel_cur = emit_ksel(t + 1)
                        pm_cur = emit_head(t + 1)
                        pg_cur = PGP.tile([128, 512], F32, tag="pg",
                                          name=f"pg{t + 1}")
                        emit_gmm(t + 1, pg_cur, ksel_cur)
                        emit_trio(pm_cur, 0)
                    # sa broadcast for the A decay (off critical)
                    MM(ph[0:128, 32:33], E_bT[:], sa8[:],
                                     start=True, stop=True)
                    sa_sb = SP.tile([128, 1], F32, tag="sasb")
                    nc.vector.tensor_copy(sa_sb[:], ph[:, 32:33])
                    emit_ph(2)
                    emit_ph(3)
                    nc.vector.tensor_copy(hT_b[:], ph[:, 0:32])
                    if not last:
                        emit_trio(pm_cur, 1)
                        emit_trio(pm_cur, 2)

                    # ---- batch-major h' update
                    tmpiv = SP.tile([8, 512], F32, tag="tmpiv")
                    nc.vector.tensor_scalar(tmpiv[:], hh[32:40, 0:512],
                                            iv40[32:40, :], None, AL.mult)
                    nc.vector.scalar_tensor_tensor(hh[0:8, 0:512],
                                                   hh[0:8, 0:512], om8[:],
                                                   tmpiv[:], AL.mult, AL.add)

                    # ---- A outer product + decay update
                    MM(pm[0:8, 256:384], ksel[:], I128[:],
                                     is_transpose=True, start=True, stop=True)
                    K_blk = SP.tile([8, 128], F32, tag="kblk")
                    nc.vector.tensor_scalar(K_blk[:], pm[0:8, 256:384], sp2[:],
                                            None, AL.mult)
                    if not last:
                        emit_trio(pm_cur, 3)
                    MM(pa[:, 0:512], K_blk[:], hh[0:8, 0:512],
                                     start=True, stop=True)
                    nc.vector.scalar_tensor_tensor(A_sb[:], A_sb[:], sa_sb[:],
                                                   pa[:], AL.mult, AL.add)
                    if not last:
                        for kc in range(4):
                            emit_dec(pm_cur, kc)
                        v1_cur, v2_cur = emit_uv(t + 1, pg_cur, sa8, sp2)

                # final recon for t = nsteps-1
                pmf = PM.tile([128, 512], F32, tag="pm")
                for kc in range(4):
                    MM(pmf[96:104, 0:256],
                                     hT_b[:, 8 * kc:8 * (kc + 1)],
                                     WdB[kc][:],
                                     start=(kc == 0), stop=(kc == 3),
                                     tile_position=(0, 96))
                rec_f = SP.tile([8, 256], F32, tag="rec", bufs=3)
                nc.vector.tensor_copy(rec_f[:], pmf[96:104, 0:256])
                nc.sync.dma_start(d_y[:, nsteps - 1, :], rec_f[:])

    nc.finalize()
    return nc


def _make_runner(nc):
    """Persistent jitted SPMD executor (mirrors bass2jax.run_bass_via_pjrt,
    but reusable across calls so the NEFF stays loaded on the devices)."""
    import jax
    from jax.experimental.shard_map import shard_map
    from jax.sharding import Mesh, PartitionSpec
    from concourse import bass2jax
    from concourse import mybir as mb

    bass2jax.install_neuronx_cc_hook()

    partition_name = (nc.partition_id_tensor.name
                      if nc.partition_id_tensor else None)
    in_names, out_names, out_avals, zero_outs = [], [], [], []
    for alloc in nc.m.functions[0].allocations:
        if not isinstance(alloc, mb.MemoryLocationSet):
            continue
        name = alloc.memorylocations[0].name
        if alloc.kind == "ExternalInput":
            if name != partition_name:
                in_names.append(name)
        elif alloc.kind == "ExternalOutput":
            out_names.append(name)
            shape = tuple(alloc.tensor_shape)
            dtype = mb.dt.np(alloc.dtype)
            out_avals.append(jax.core.ShapedArray(shape, dtype))
            zero_outs.append(np.zeros(shape, dtype))
    n_params = len(in_names)
    n_outs = len(out_avals)
    all_in_names = list(in_names) + list(out_names)
    if partition_name is not None:
        all_in_names.append(partition_name)

    def _body(*args):
        operands = list(args)
        if partition_name is not None:
            operands.append(bass2jax.partition_id_tensor())
        outs = bass2jax._bass_exec_p.bind(
            *operands,
            out_avals=tuple(out_avals),
            in_names=tuple(all_in_names),
            out_names=tuple(out_names),
            lowering_input_output_aliases=(),
            sim_require_finite=True,
            sim_require_nnan=True,
            nc=nc,
        )
        return tuple(outs)

    devices = jax.devices()[:NCORES]
    mesh = Mesh(np.asarray(devices), ("core",))
    in_specs = (PartitionSpec("core"),) * (n_params + n_outs)
    out_specs = (PartitionSpec("core"),) * len(out_names)
    donate = tuple(range(n_params, n_params + n_outs))
    sharded = jax.jit(
        shard_map(_body, mesh=mesh, in_specs=in_specs, out_specs=out_specs,
                  check_rep=False),
        donate_argnums=donate, keep_unused=True)

    def run(in_maps):
        concat_in = [
            np.concatenate([np.asarray(in_maps[c][n]) for c in range(NCORES)],
                           axis=0)
            for n in in_names
        ]
        concat_zeros = [
            np.zeros((NCORES * z.shape[0], *z.shape[1:]), z.dtype)
            for z in zero_outs
        ]
        out_arrs = sharded(*concat_in, *concat_zeros)
        return [
            {n: np.asarray(out_arrs[i]).reshape(NCORES, *out_avals[i].shape)[c]
             for i, n in enumerate(out_names)}
            for c in range(NCORES)
        ]

    return run


def kernel(x, W_in, W_rec, W_pred, B_proj, W_dec, b_dec, _nsteps=T, _trace=False):
    x = np.asarray(x, np.float32)
    consts = _host_constants(np.asarray(W_in, np.float32),
                             np.asarray(W_rec, np.float32),
                             np.asarray(W_pred, np.float32),
                             np.asarray(B_proj, np.float32),
                             np.asarray(W_dec, np.float32),
                             np.asarray(b_dec, np.float32))
    key = _nsteps
    if key not in _CACHE:
        nc = _build(_nsteps)
        _CACHE[key] = (nc, _make_runner(nc))
    nc, run = _CACHE[key]

    in_maps = []
    for c in range(NCORES):
        xs = x[c * BL:(c + 1) * BL]                      # [8, T, D]
        xT = np.ascontiguousarray(
            xs.transpose(2, 1, 0).reshape(2, 128, T, BL).transpose(1, 0, 2, 3))
        m = dict(consts)
        m["xT"] = xT
        m["nxT"] = np.ascontiguousarray(-xT)
        in_maps.append(m)

    if _trace:
        import os
        import types
        import concourse.bass_utils as bu
        if "antenv.axon_hooks" not in sys.modules:
            mod = types.ModuleType("antenv.axon_hooks")
            mod._hook = None
            mod.set_axon_ntff_profile_hook = lambda h: setattr(mod, "_hook", h)
            mod.get_axon_ntff_profile_hook = lambda: mod._hook
            sys.modules["antenv.axon_hooks"] = mod
            from trn_agent_boot.trn_boot import _ntff_profile_via_ctypes
            mod._hook = _ntff_profile_via_ctypes("/opt/axon/libaxon_pjrt.so")
        bu.upload_artifacts = lambda tmpdir: "local://" + tmpdir
        tmpdir = os.environ.get("TRACE_DIR") or None
        res = run_bass_kernel_spmd(nc, in_maps, core_ids=list(range(NCORES)),
                                   trace=True, tmpdir=tmpdir)
        kernel.last_exec_time_ns = res.exec_time_ns
        kernel.last_results = res
        results = res.results
        y = np.concatenate([results[c]["y"] for c in range(NCORES)], axis=0)
        bd = np.asarray(b_dec, np.float32)
        if np.any(bd):
            y = y + bd[None, None, :]
        return y

    results = run(in_maps)
    y = np.concatenate([results[c]["y"] for c in range(NCORES)], axis=0)
    bd = np.asarray(b_dec, np.float32)
    if np.any(bd):
        y = y + bd[None, None, :]
    return y

